# revision 31
# baseline (speedup 1.0000x reference)
"""BallPooling (stride=2) Trainium2 Bass kernel, data-parallel over 8 NeuronCores.

Layout strategy (per core, 32768 leaves = 16384 balls):
  - 16 super-tiles of 1024 balls; within a super-tile, partition p holds the 8
    consecutive balls  st*1024 + p*8 + t  (t = subtile index 0..8).  Every DMA
    is therefore long-contiguous per partition (mv: 16KB runs).
  - The host pre-permutes each ball's 512 mv values from (i, y) to
    (y-quarter q, i, tq) order, so each of the 4 PE transposes per 128-ball
    subtile reads a contiguous 128-col slice and yields a K-chunk whose
    matmul writes a disjoint contiguous 64-col PSUM block in (y, o) layout —
    1 output pass instead of 4 accumulating passes (fp32 matmuls cost 4
    cycles/output-row, so avoiding re-accumulation is the big PE win).
  - The transposed sc_cat (32 sc channels + rel-dist row + ones row for the
    bias) shares the per-subtile 2-bank PSUM tile; one copy moves all 640
    transposed columns to SBUF (copies alternate ScalarE/VectorE to balance).
  - Two extra matmuls fold the scalar path into the same PSUM tile: grade-0
    adds at cols 0..16 ((y=0, o) block) and sc_out at cols 256..272.
  - EquiLayerNorm: ACT Square+accum_out for the mv sum-of-squares, DVE
    bn_stats/bn_aggr for the scalar LN, rsqrt batched per 3-subtile group
    (ACT Sqrt + DVE reciprocal), then per-partition-scalar scale ops write
    SBUF staging; 3 large DMAs out per super-tile.
  - batch_out = batch_idx[::2] handled as an int32-pair gather on DVE.
"""

import json
import numpy as np

P = 128          # partitions
T = 8            # subtiles (of 128 balls) per super-tile
ST = 16          # super-tiles per core
NCORES = 8
NB_CORE = ST * T * P          # balls per core  (16384)
NL_CORE = 2 * NB_CORE         # leaves per core (32768)
EPS = 1e-5
GRADE = np.array([0, 1, 1, 1, 1, 2, 2, 2, 2, 2, 2, 3, 3, 3, 3, 4])
GROUPS = [(0, 3), (3, 6), (6, 8)]   # batched factor math; PSUM bufs=4 > group size


def _split_waits_json(bir_bytes: bytes, max_waits: int = 1) -> bytes:
    """This toolchain's walrus rejects instructions carrying more than one
    semaphore wait ("Too many sync wait commands").  Split extra waits into
    standalone single-wait EventSemaphore instructions on the same engine."""
    j = json.loads(bir_bytes)
    counter = [0]

    def walk(block):
        insts = block.get("instructions")
        if insts:
            new = []
            for inst in insts:
                si = inst.get("sync_info")
                ow = (si or {}).get("on_wait") or []
                if len(ow) > max_waits:
                    for w in ow[:-max_waits]:
                        counter[0] += 1
                        new.append({
                            "debug": inst.get("debug", 0),
                            "engine": inst.get("engine"),
                            "ins": [],
                            "name": f"WSPLIT-{counter[0]}",
                            "opcode": "EventSemaphore",
                            "outs": [],
                            "sync_info": {"on_update": [], "on_wait": [w]},
                        })
                    si["on_wait"] = ow[-max_waits:]
                new.append(inst)
            block["instructions"] = new
        for sub in block.get("blocks") or []:
            walk(sub)

    for fn in j["functions"]:
        for b in fn["blocks"]:
            walk(b)
    return json.dumps(j).encode()


def _patch_bass(nc):
    orig = nc.to_json_bytes

    def patched(*a, **k):
        return _split_waits_json(orig(*a, **k))

    nc.to_json_bytes = patched
    return nc


def prep_weights(w_mv, w_s2mv, w_mv2s, w_s, b_s):
    """Host-side weight rearrangement.

    Wmm[j] : [K=128, 272] for mv chunk j (i in [8j,8j+8), all y).
      K index k = il*16 + y  (il = i-8j), matching the PE transpose of the
      contiguous 128-col slice of the natural [ball, i*16+y] layout.
      cols 0..256   : c = y*16 + o   ->  w_full[o, i, y]   (block diagonal in y)
      cols 256..272 : c = 256 + o    ->  [y==0] * w_mv2s[o, i]
    Wsc : [34, 32] for the transposed sc_cat (rows: 32 sc channels, d-row,
      ones-row).  cols 0..16 -> mv grade-0 (psum cols 0..16, y=0 block);
      cols 16..32 -> sc_out (psum cols 256..272).
    """
    O, I = 16, 32
    w_full = w_mv[:, :, GRADE]                      # (16, 32, 16) [o, i, y]
    # y-quarter chunking: chunk q covers y in [4q, 4q+4), all 32 i; K index
    # k = i*4 + tq (tq = y-4q).  Chunk q's output block is psum cols
    # [q*64, (q+1)*64) with local col o*4 + tq  (global col o*16 + y).
    wmm = np.zeros((4, 128, 64), np.float32)
    for q in range(4):
        for i in range(32):
            for tq in range(4):
                y = 4 * q + tq
                wmm[q, i * 4 + tq, tq * 16:tq * 16 + 16] = w_full[:, i, y]  # local (tq,o)
    # mv->scalar weights ride on chunk 0's lhsT (y=0 rows are k = i*4):
    w2s = np.zeros((128, 16), np.float32)
    for i in range(32):
        w2s[i * 4, :] = w_mv2s[:, i]
    wsc = np.zeros((34, 32), np.float32)
    wsc[0:32, 0:16] = w_s2mv[:, 0:32].T
    wsc[32, 0:16] = w_s2mv[:, 32] + w_s2mv[:, 33]
    wsc[0:32, 16:32] = w_s[:, 0:32].T
    wsc[32, 16:32] = w_s[:, 32] + w_s[:, 33]
    wsc[33, 16:32] = b_s
    return wmm, w2s, wsc


def build_nc(repeat=1):
    import concourse.bass as bass
    import concourse.tile as tile
    from concourse import mybir
    from concourse.masks import make_identity

    f32 = mybir.dt.float32
    f32r = mybir.dt.float32r
    i32 = mybir.dt.int32
    AF = mybir.ActivationFunctionType
    OP = mybir.AluOpType
    AX = mybir.AxisListType

    nc = bass.Bass("TRN2")
    mv = nc.dram_tensor("mv", (NB_CORE, 512), f32, kind="ExternalInput")
    sc = nc.dram_tensor("sc", (NB_CORE, 32), f32, kind="ExternalInput")
    pos = nc.dram_tensor("pos", (NB_CORE, 6), f32, kind="ExternalInput")
    wmm = nc.dram_tensor("wmm", (4, 128, 64), f32, kind="ExternalInput")
    w2s = nc.dram_tensor("w2s", (128, 16), f32, kind="ExternalInput")
    wsc = nc.dram_tensor("wsc", (34, 32), f32, kind="ExternalInput")
    bidx = nc.dram_tensor("bidx", (128, 512), i32, kind="ExternalInput")
    mvn = nc.dram_tensor("mvn", (NB_CORE, 256), f32, kind="ExternalOutput")
    scn = nc.dram_tensor("scn", (NB_CORE, 16), f32, kind="ExternalOutput")
    cen = nc.dram_tensor("cen", (NB_CORE, 3), f32, kind="ExternalOutput")
    bout = nc.dram_tensor("bout", (128, 256), i32, kind="ExternalOutput")

    mv_ap = mv.rearrange("(st p t) m -> st p t m", p=P, t=T)
    sc_ap = sc.rearrange("(st p t) m -> st p t m", p=P, t=T)
    pos_ap = pos.rearrange("(st p t) m -> st p t m", p=P, t=T)
    mvn_ap = mvn.rearrange("(st p t) m -> st p t m", p=P, t=T)
    scn_ap = scn.rearrange("(st p t) m -> st p t m", p=P, t=T)
    cen_ap = cen.rearrange("(st p t) m -> st p t m", p=P, t=T)

    with tile.TileContext(nc) as tc:
        with (
            tc.tile_pool(name="consts", bufs=1) as consts,
            tc.tile_pool(name="mvin", bufs=3) as mvin,
            tc.tile_pool(name="scin", bufs=3) as scin,
            tc.tile_pool(name="posin", bufs=3) as posin,
            tc.tile_pool(name="mvt", bufs=5) as mvtp,
                        tc.tile_pool(name="stats", bufs=3) as stats,
            tc.tile_pool(name="stage", bufs=3) as stage_p,
            tc.tile_pool(name="scr", bufs=8) as scr,
            tc.tile_pool(name="sqp", bufs=3) as sqp,
            tc.tile_pool(name="tpsum", bufs=2, space="PSUM") as tpsum,
            tc.tile_pool(name="opsum", bufs=4, space="PSUM") as opsum,
        ):
            ident = consts.tile([128, 128], f32)
            make_identity(nc, ident)
            eps_t = consts.tile([128, 1], f32)
            nc.vector.memset(eps_t, EPS)
            w_sb = consts.tile([128, 4, 64], f32)
            nc.sync.dma_start(out=w_sb, in_=wmm.rearrange("j k c -> k j c"))
            w2s_sb = consts.tile([128, 16], f32)
            nc.sync.dma_start(out=w2s_sb, in_=w2s[:, :])
            wsc_sb = consts.tile([34, 32], f32)
            nc.sync.dma_start(out=wsc_sb, in_=wsc[:, :])

            # ---- batch_out = batch_idx[::2] (int32 pair gather) ----
            bt = consts.tile([128, 512], i32)
            nc.sync.dma_start(out=bt, in_=bidx[:, :])
            bo = consts.tile([128, 256], i32)
            nc.vector.tensor_copy(
                out=bo.rearrange("p (b f) -> p b f", f=2),
                in_=bt.rearrange("p (b f) -> p b f", f=4)[:, :, 0:2],
            )
            nc.sync.dma_start(out=bout[:, :], in_=bo)

            for st in [s for _ in range(repeat) for s in range(ST)]:
                mv_t = mvin.tile([P, T, 512], f32)
                nc.sync.dma_start(out=mv_t, in_=mv_ap[st])
                sccat = scin.tile([P, T, 34], f32)
                nc.sync.dma_start(out=sccat[:, :, 0:32], in_=sc_ap[st])
                pos_t = posin.tile([P, T, 6], f32)
                nc.sync.dma_start(out=pos_t, in_=pos_ap[st])

                # ---- geometry: centers + rel-dist (batched over all T) ----
                cen_t = stage_p.tile([P, T, 3], f32, tag="cen")
                diff = scr.tile([P, T, 3], f32, tag="diff")
                nc.vector.tensor_sub(out=diff, in0=pos_t[:, :, 0:3], in1=pos_t[:, :, 3:6])
                nc.vector.tensor_add(out=cen_t, in0=pos_t[:, :, 0:3], in1=pos_t[:, :, 3:6])
                nc.vector.tensor_scalar_mul(out=cen_t, in0=cen_t, scalar1=0.5)
                dsum = scr.tile([P, T], f32, tag="dsum")
                nc.vector.tensor_mul(out=diff, in0=diff, in1=diff)
                nc.vector.reduce_sum(out=dsum, in_=diff, axis=AX.X)
                # d = sqrt(0.25 * sum(diff^2)); both leaves share it
                nc.scalar.activation(
                    out=sccat[:, :, 32:33].rearrange("p t o -> p (t o)"),
                    in_=dsum, func=AF.Sqrt, scale=0.25)
                nc.gpsimd.memset(sccat[:, :, 33:34], 1.0)

                # ---- per-supertile stats tiles ----
                mvss = stats.tile([P, T], f32, tag="mvss")
                bnmv = stats.tile([P, T, 2], f32, tag="bnmv")
                fac = stats.tile([P, T], f32, tag="fac")
                rstd = stats.tile([P, T], f32, tag="rstd")
                stage = stage_p.tile([P, T, 256], f32, tag="mvstage")
                scstage = stage_p.tile([P, T, 16], f32, tag="scstage")

                sub_sb = {}

                def prep_subtile(tt):
                    # all transposes for one subtile into one 2-bank psum tile:
                    # cols 0..512 = 4 y-quarter mvT chunks, cols 512..640 rows
                    # 0..34 = transposed sc_cat.  One copy moves it all to SBUF.
                    t_ps = tpsum.tile([128, 640], f32, tag="t_ps")
                    for q in range(4):
                        nc.tensor.transpose(
                            t_ps[:, q * 128:(q + 1) * 128],
                            mv_t[:, tt, q * 128:(q + 1) * 128], ident)
                    nc.tensor.transpose(t_ps[0:34, 512:640], sccat[:, tt, :], ident)
                    t_sb = mvtp.tile([128, 640], f32)
                    if tt % 8 in (1, 4, 6):
                        nc.vector.tensor_copy(out=t_sb, in_=t_ps)
                    else:
                        nc.scalar.copy(out=t_sb, in_=t_ps)
                    sub_sb[tt] = t_sb

                def do_subtile(t):
                    t_sb = sub_sb.pop(t)
                    ops_t = opsum.tile([128, 272], f32, tag="ops")
                    for q in range(4):
                        nc.tensor.matmul(
                            ops_t[:, q * 64:(q + 1) * 64],
                            t_sb[:, q * 128:(q + 1) * 128],
                            w_sb[:, q, :], start=(q == 0), stop=False)
                    nc.tensor.matmul(
                        ops_t[:, 256:272], t_sb[:, 0:128],
                        w2s_sb, start=False, stop=False)
                    sct_h = t_sb[0:34, 512:640]
                    nc.tensor.matmul(ops_t[:, 0:16], sct_h, wsc_sb[:, 0:16],
                                     start=False, stop=False)
                    nc.tensor.matmul(ops_t[:, 256:272], sct_h, wsc_sb[:, 16:32],
                                     start=False, stop=True)
                    # stats: sum of squares over all 256 mv comps; bn stats on sc
                    sq_t = sqp.tile([128, 256], f32, tag="sq")
                    nc.scalar.activation(
                        out=sq_t, in_=ops_t[:, 0:256], func=AF.Square,
                        accum_out=mvss[:, t:t + 1])
                    bnst = scr.tile([P, 6], f32, tag="bnst")
                    nc.vector.bn_stats(out=bnst, in_=ops_t[:, 256:272])
                    nc.vector.bn_aggr(out=bnmv[:, t, :], in_=bnst)
                    return ops_t

                def finish_subtile(t, ops_t):
                    stv = stage[:, t, :].rearrange("p (o y) -> p y o", o=16, y=16)
                    opv = ops_t[:, 0:256].rearrange("p (y o) -> p y o", y=16, o=16)
                    nc.vector.tensor_scalar_mul(
                        out=stv, in0=opv, scalar1=fac[:, t:t + 1])
                    nc.vector.tensor_scalar(
                        out=scstage[:, t, :], in0=ops_t[:, 256:272],
                        scalar1=bnmv[:, t, 0:1],
                        scalar2=rstd[:, t:t + 1],
                        op0=OP.subtract, op1=OP.mult)

                for (g0, g1) in GROUPS:
                    for t in range(g0, g1):
                        prep_subtile(t)
                    live = []
                    for t in range(g0, g1):
                        live.append((t, do_subtile(t)))
                    # batched factor math for the group
                    nc.scalar.activation(out=fac[:, g0:g1], in_=mvss[:, g0:g1],
                                         func=AF.Sqrt, scale=1.0 / 16, bias=eps_t)
                    nc.vector.reciprocal(out=fac[:, g0:g1], in_=fac[:, g0:g1])
                    nc.scalar.activation(
                        out=rstd[:, g0:g1],
                        in_=bnmv[:, g0:g1, 1:2].rearrange("p t o -> p (t o)"),
                        func=AF.Sqrt, bias=eps_t)
                    nc.vector.reciprocal(out=rstd[:, g0:g1], in_=rstd[:, g0:g1])
                    for (t, ops_t) in live:
                        finish_subtile(t, ops_t)

                nc.sync.dma_start(out=mvn_ap[st], in_=stage)
                nc.sync.dma_start(out=scn_ap[st], in_=scstage)
                nc.sync.dma_start(out=cen_ap[st], in_=cen_t)

    _patch_bass(nc)
    return nc


_NC_CACHE = None


def _get_nc():
    global _NC_CACHE
    if _NC_CACHE is None:
        _NC_CACHE = build_nc()
    return _NC_CACHE


def make_in_maps(mv, sc, pos, w_mv, w_s2mv, w_mv2s, w_s, b_s, batch_idx):
    wmm, w2s, wsc = prep_weights(
        np.asarray(w_mv, np.float32), np.asarray(w_s2mv, np.float32),
        np.asarray(w_mv2s, np.float32), np.asarray(w_s, np.float32),
        np.asarray(b_s, np.float32))
    mv = np.ascontiguousarray(mv, np.float32).reshape(-1, 256)
    sc = np.ascontiguousarray(sc, np.float32)
    pos = np.ascontiguousarray(pos, np.float32)
    bidx = np.ascontiguousarray(batch_idx).astype(np.int64, copy=False)
    in_maps = []
    for c in range(NCORES):
        L = slice(c * NL_CORE, (c + 1) * NL_CORE)
        in_maps.append({
            "mv": np.ascontiguousarray(
                mv[L].reshape(NB_CORE, 32, 4, 4).transpose(0, 2, 1, 3)
            ).reshape(NB_CORE, 512),
            "sc": np.ascontiguousarray(sc[L]).reshape(NB_CORE, 32),
            "pos": np.ascontiguousarray(pos[L]).reshape(NB_CORE, 6),
            "wmm": wmm,
            "w2s": w2s,
            "wsc": wsc,
            "bidx": np.ascontiguousarray(bidx[L]).view(np.int32).reshape(128, 512),
        })
    return in_maps


def assemble(results):
    mv_n = np.concatenate(
        [r["mvn"].reshape(NB_CORE, 16, 16) for r in results], axis=0)
    sc_n = np.concatenate([r["scn"] for r in results], axis=0)
    centers = np.concatenate([r["cen"] for r in results], axis=0)
    batch_out = np.concatenate(
        [np.ascontiguousarray(r["bout"]).reshape(-1).view(np.int64)
         for r in results], axis=0)
    return mv_n, sc_n, centers, batch_out


def kernel(mv, sc, pos, w_mv, w_s2mv, w_mv2s, w_s, b_s, batch_idx):
    from concourse.bass_utils import run_bass_kernel_spmd

    nc = _get_nc()
    in_maps = make_in_maps(mv, sc, pos, w_mv, w_s2mv, w_mv2s, w_s, b_s, batch_idx)
    res = run_bass_kernel_spmd(nc, in_maps, core_ids=list(range(NCORES)))
    return assemble(res.results)


# revision 33
# speedup vs baseline: 2.8110x; 2.8110x over previous
"""BallPooling (stride=2) Trainium2 Bass kernel, data-parallel over 8 NeuronCores.

Layout strategy (per core, 32768 leaves = 16384 balls):
  - 16 super-tiles of 1024 balls; within a super-tile, partition p holds the 8
    consecutive balls  st*1024 + p*8 + t  (t = subtile index 0..8).  Every DMA
    is therefore long-contiguous per partition (mv: 16KB runs).
  - The host pre-permutes each ball's 512 mv values from (i, y) to
    (y-quarter q, i, tq) order, so each of the 4 PE transposes per 128-ball
    subtile reads a contiguous 128-col slice and yields a K-chunk whose
    matmul writes a disjoint contiguous 64-col PSUM block in (y, o) layout —
    1 output pass instead of 4 accumulating passes (fp32 matmuls cost 4
    cycles/output-row, so avoiding re-accumulation is the big PE win).
  - The transposed sc_cat (32 sc channels + rel-dist row + ones row for the
    bias) shares the per-subtile 2-bank PSUM tile; one copy moves all 640
    transposed columns to SBUF (copies alternate ScalarE/VectorE to balance).
  - Two extra matmuls fold the scalar path into the same PSUM tile: grade-0
    adds at cols 0..16 ((y=0, o) block) and sc_out at cols 256..272.
  - EquiLayerNorm: ACT Square+accum_out for the mv sum-of-squares, DVE
    bn_stats/bn_aggr for the scalar LN, rsqrt batched per 3-subtile group
    (ACT Sqrt + DVE reciprocal), then per-partition-scalar scale ops write
    SBUF staging; 3 large DMAs out per super-tile.
  - batch_out = batch_idx[::2] handled as an int32-pair gather on DVE.
"""

import json
import numpy as np

P = 128          # partitions
T = 8            # subtiles (of 128 balls) per super-tile
ST = 16          # super-tiles per core
NCORES = 8
NB_CORE = ST * T * P          # balls per core  (16384)
NL_CORE = 2 * NB_CORE         # leaves per core (32768)
EPS = 1e-5
GRADE = np.array([0, 1, 1, 1, 1, 2, 2, 2, 2, 2, 2, 3, 3, 3, 3, 4])
GROUPS = [(0, 3), (3, 6), (6, 8)]   # batched factor math; PSUM bufs=4 > group size


def _split_waits_json(bir_bytes: bytes, max_waits: int = 1) -> bytes:
    """This toolchain's walrus rejects instructions carrying more than one
    semaphore wait ("Too many sync wait commands").  Split extra waits into
    standalone single-wait EventSemaphore instructions on the same engine."""
    j = json.loads(bir_bytes)
    counter = [0]

    def walk(block):
        insts = block.get("instructions")
        if insts:
            new = []
            for inst in insts:
                si = inst.get("sync_info")
                ow = (si or {}).get("on_wait") or []
                if len(ow) > max_waits:
                    for w in ow[:-max_waits]:
                        counter[0] += 1
                        new.append({
                            "debug": inst.get("debug", 0),
                            "engine": inst.get("engine"),
                            "ins": [],
                            "name": f"WSPLIT-{counter[0]}",
                            "opcode": "EventSemaphore",
                            "outs": [],
                            "sync_info": {"on_update": [], "on_wait": [w]},
                        })
                    si["on_wait"] = ow[-max_waits:]
                new.append(inst)
            block["instructions"] = new
        for sub in block.get("blocks") or []:
            walk(sub)

    for fn in j["functions"]:
        for b in fn["blocks"]:
            walk(b)
    return json.dumps(j).encode()


def _patch_bass(nc):
    orig = nc.to_json_bytes

    def patched(*a, **k):
        return _split_waits_json(orig(*a, **k))

    nc.to_json_bytes = patched
    return nc


def prep_weights(w_mv, w_s2mv, w_mv2s, w_s, b_s):
    """Host-side weight rearrangement for y-quarter K-chunks.

    K index within chunk q: k = i*4 + tq (tq = y-4q), matching the PE
    transpose of the host-permuted (q, i, tq) mv layout.  PSUM col layout:
    0..16 = sc_out, 16 + y*16 + o = mv (y, o).  Chunk 0's matmul also carries
    the w_mv2s columns (sc block); the single sc_cat matmul carries w_s,
    the rel-dist row (w[:,32]+w[:,33] folded), the bias row, and w_s2mv.
    """
    O, I = 16, 32
    w_full = w_mv[:, :, GRADE]                      # (16, 32, 16) [o, i, y]
    # y-quarter chunking: chunk q covers y in [4q, 4q+4), all 32 i; K index
    # k = i*4 + tq (tq = y-4q).  Chunk q's output block is psum cols
    # [q*64, (q+1)*64) with local col o*4 + tq  (global col o*16 + y).
    # chunk 0 carries [w_mv2s -> sc cols 0..16 | its 64 mv cols]; chunks 1-3
    # carry only their 64 mv cols.  PSUM col layout: 0..16 = sc_out,
    # 16 + y*16 + o = mv (y, o).
    wmm0 = np.zeros((128, 80), np.float32)
    wmm = np.zeros((3, 128, 64), np.float32)
    for q in range(4):
        for i in range(32):
            for tq in range(4):
                y = 4 * q + tq
                if q == 0:
                    wmm0[i * 4 + tq, 16 + tq * 16:16 + tq * 16 + 16] = w_full[:, i, y]
                else:
                    wmm[q - 1, i * 4 + tq, tq * 16:tq * 16 + 16] = w_full[:, i, y]
    for i in range(32):
        wmm0[i * 4, 0:16] = w_mv2s[:, i]
    wsc = np.zeros((34, 32), np.float32)
    wsc[0:32, 0:16] = w_s[:, 0:32].T
    wsc[32, 0:16] = w_s[:, 32] + w_s[:, 33]
    wsc[33, 0:16] = b_s
    wsc[0:32, 16:32] = w_s2mv[:, 0:32].T
    wsc[32, 16:32] = w_s2mv[:, 32] + w_s2mv[:, 33]
    return wmm0, wmm, wsc


def build_nc(repeat=1):
    import concourse.bass as bass
    import concourse.tile as tile
    from concourse import mybir
    from concourse.masks import make_identity

    f32 = mybir.dt.float32
    f32r = mybir.dt.float32r
    i32 = mybir.dt.int32
    AF = mybir.ActivationFunctionType
    OP = mybir.AluOpType
    AX = mybir.AxisListType

    nc = bass.Bass("TRN2")
    mv = nc.dram_tensor("mv", (NB_CORE, 512), f32, kind="ExternalInput")
    sc = nc.dram_tensor("sc", (NB_CORE, 32), f32, kind="ExternalInput")
    pos = nc.dram_tensor("pos", (NB_CORE, 6), f32, kind="ExternalInput")
    wmm0 = nc.dram_tensor("wmm0", (128, 80), f32, kind="ExternalInput")
    wmm = nc.dram_tensor("wmm", (3, 128, 64), f32, kind="ExternalInput")
    wsc = nc.dram_tensor("wsc", (34, 32), f32, kind="ExternalInput")
    bidx = nc.dram_tensor("bidx", (128, 512), i32, kind="ExternalInput")
    mvn = nc.dram_tensor("mvn", (NB_CORE, 256), f32, kind="ExternalOutput")
    scn = nc.dram_tensor("scn", (NB_CORE, 16), f32, kind="ExternalOutput")
    cen = nc.dram_tensor("cen", (NB_CORE, 3), f32, kind="ExternalOutput")
    bout = nc.dram_tensor("bout", (128, 256), i32, kind="ExternalOutput")

    mv_ap = mv.rearrange("(st p t) m -> st p t m", p=P, t=T)
    sc_ap = sc.rearrange("(st p t) m -> st p t m", p=P, t=T)
    pos_ap = pos.rearrange("(st p t) m -> st p t m", p=P, t=T)
    mvn_ap = mvn.rearrange("(st p t) m -> st p t m", p=P, t=T)
    scn_ap = scn.rearrange("(st p t) m -> st p t m", p=P, t=T)
    cen_ap = cen.rearrange("(st p t) m -> st p t m", p=P, t=T)

    with tile.TileContext(nc) as tc:
        with (
            tc.tile_pool(name="consts", bufs=1) as consts,
            tc.tile_pool(name="mvin", bufs=3) as mvin,
            tc.tile_pool(name="scin", bufs=3) as scin,
            tc.tile_pool(name="posin", bufs=3) as posin,
            tc.tile_pool(name="mvt", bufs=5) as mvtp,
                        tc.tile_pool(name="stats", bufs=3) as stats,
            tc.tile_pool(name="stage", bufs=3) as stage_p,
            tc.tile_pool(name="scr", bufs=8) as scr,
            tc.tile_pool(name="sqp", bufs=3) as sqp,
            tc.tile_pool(name="tpsum", bufs=2, space="PSUM") as tpsum,
            tc.tile_pool(name="opsum", bufs=4, space="PSUM") as opsum,
        ):
            ident = consts.tile([128, 128], f32)
            make_identity(nc, ident)
            eps_t = consts.tile([128, 1], f32)
            nc.vector.memset(eps_t, EPS)
            w0_sb = consts.tile([128, 80], f32)
            nc.sync.dma_start(out=w0_sb, in_=wmm0[:, :])
            w_sb = consts.tile([128, 3, 64], f32)
            nc.sync.dma_start(out=w_sb, in_=wmm.rearrange("j k c -> k j c"))
            wsc_sb = consts.tile([34, 32], f32)
            nc.sync.dma_start(out=wsc_sb, in_=wsc[:, :])

            # ---- batch_out = batch_idx[::2] (int32 pair gather) ----
            bt = consts.tile([128, 512], i32)
            nc.sync.dma_start(out=bt, in_=bidx[:, :])
            bo = consts.tile([128, 256], i32)
            nc.vector.tensor_copy(
                out=bo.rearrange("p (b f) -> p b f", f=2),
                in_=bt.rearrange("p (b f) -> p b f", f=4)[:, :, 0:2],
            )
            nc.sync.dma_start(out=bout[:, :], in_=bo)

            for st in [s for _ in range(repeat) for s in range(ST)]:
                mv_t = mvin.tile([P, T, 512], f32)
                nc.sync.dma_start(out=mv_t, in_=mv_ap[st])
                sccat = scin.tile([P, T, 34], f32)
                nc.sync.dma_start(out=sccat[:, :, 0:32], in_=sc_ap[st])
                pos_t = posin.tile([P, T, 6], f32)
                nc.sync.dma_start(out=pos_t, in_=pos_ap[st])

                # ---- geometry: centers + rel-dist (batched over all T) ----
                cen_t = stage_p.tile([P, T, 3], f32, tag="cen")
                diff = scr.tile([P, T, 3], f32, tag="diff")
                nc.vector.tensor_sub(out=diff, in0=pos_t[:, :, 0:3], in1=pos_t[:, :, 3:6])
                nc.vector.tensor_add(out=cen_t, in0=pos_t[:, :, 0:3], in1=pos_t[:, :, 3:6])
                nc.vector.tensor_scalar_mul(out=cen_t, in0=cen_t, scalar1=0.5)
                dsum = scr.tile([P, T], f32, tag="dsum")
                nc.vector.tensor_mul(out=diff, in0=diff, in1=diff)
                nc.vector.reduce_sum(out=dsum, in_=diff, axis=AX.X)
                # d = sqrt(0.25 * sum(diff^2)); both leaves share it
                nc.scalar.activation(
                    out=sccat[:, :, 32:33].rearrange("p t o -> p (t o)"),
                    in_=dsum, func=AF.Sqrt, scale=0.25)
                nc.gpsimd.memset(sccat[:, :, 33:34], 1.0)

                # ---- per-supertile stats tiles ----
                mvss = stats.tile([P, T], f32, tag="mvss")
                bnmv = stats.tile([P, T, 2], f32, tag="bnmv")
                fac = stats.tile([P, T], f32, tag="fac")
                rstd = stats.tile([P, T], f32, tag="rstd")
                stage = stage_p.tile([P, T, 256], f32, tag="mvstage")
                scstage = stage_p.tile([P, T, 16], f32, tag="scstage")

                sub_sb = {}

                def prep_subtile(tt):
                    # all transposes for one subtile into one 2-bank psum tile:
                    # cols 0..512 = 4 y-quarter mvT chunks, cols 512..640 rows
                    # 0..34 = transposed sc_cat.  One copy moves it all to SBUF.
                    t_ps = tpsum.tile([128, 640], f32, tag="t_ps")
                    for q in range(4):
                        nc.tensor.transpose(
                            t_ps[:, q * 128:(q + 1) * 128],
                            mv_t[:, tt, q * 128:(q + 1) * 128], ident)
                    nc.tensor.transpose(t_ps[0:34, 512:640], sccat[:, tt, :], ident)
                    t_sb = mvtp.tile([128, 640], f32)
                    if tt % 8 in (1, 4, 6):
                        nc.vector.tensor_copy(out=t_sb, in_=t_ps)
                    else:
                        nc.scalar.copy(out=t_sb, in_=t_ps)
                    sub_sb[tt] = t_sb

                def do_subtile(t):
                    t_sb = sub_sb.pop(t)
                    ops_t = opsum.tile([128, 272], f32, tag="ops")
                    nc.tensor.matmul(ops_t[:, 0:80], t_sb[:, 0:128], w0_sb,
                                     start=True, stop=False)
                    for q in range(1, 4):
                        nc.tensor.matmul(
                            ops_t[:, 16 + q * 64:16 + (q + 1) * 64],
                            t_sb[:, q * 128:(q + 1) * 128],
                            w_sb[:, q - 1, :], start=False, stop=False)
                    sct_h = t_sb[0:34, 512:640]
                    nc.tensor.matmul(ops_t[:, 0:32], sct_h, wsc_sb,
                                     start=False, stop=True)
                    # stats: sum of squares over all 256 mv comps; bn stats on sc
                    sq_t = sqp.tile([128, 256], f32, tag="sq")
                    nc.scalar.activation(
                        out=sq_t, in_=ops_t[:, 16:272], func=AF.Square,
                        accum_out=mvss[:, t:t + 1])
                    bnst = scr.tile([P, 6], f32, tag="bnst")
                    nc.vector.bn_stats(out=bnst, in_=ops_t[:, 0:16])
                    nc.vector.bn_aggr(out=bnmv[:, t, :], in_=bnst)
                    return ops_t

                def finish_subtile(t, ops_t):
                    stv = stage[:, t, :].rearrange("p (o y) -> p y o", o=16, y=16)
                    opv = ops_t[:, 16:272].rearrange("p (y o) -> p y o", y=16, o=16)
                    nc.vector.tensor_scalar_mul(
                        out=stv, in0=opv, scalar1=fac[:, t:t + 1])
                    nc.vector.tensor_scalar(
                        out=scstage[:, t, :], in0=ops_t[:, 0:16],
                        scalar1=bnmv[:, t, 0:1],
                        scalar2=rstd[:, t:t + 1],
                        op0=OP.subtract, op1=OP.mult)

                for (g0, g1) in GROUPS:
                    for t in range(g0, g1):
                        prep_subtile(t)
                    live = []
                    for t in range(g0, g1):
                        live.append((t, do_subtile(t)))
                    # batched factor math for the group
                    nc.scalar.activation(out=fac[:, g0:g1], in_=mvss[:, g0:g1],
                                         func=AF.Sqrt, scale=1.0 / 16, bias=eps_t)
                    nc.vector.reciprocal(out=fac[:, g0:g1], in_=fac[:, g0:g1])
                    nc.scalar.activation(
                        out=rstd[:, g0:g1],
                        in_=bnmv[:, g0:g1, 1:2].rearrange("p t o -> p (t o)"),
                        func=AF.Sqrt, bias=eps_t)
                    nc.vector.reciprocal(out=rstd[:, g0:g1], in_=rstd[:, g0:g1])
                    for (t, ops_t) in live:
                        finish_subtile(t, ops_t)

                nc.sync.dma_start(out=mvn_ap[st], in_=stage)
                nc.sync.dma_start(out=scn_ap[st], in_=scstage)
                nc.sync.dma_start(out=cen_ap[st], in_=cen_t)

    _patch_bass(nc)
    return nc


_NC_CACHE = None


def _get_nc():
    global _NC_CACHE
    if _NC_CACHE is None:
        _NC_CACHE = build_nc()
    return _NC_CACHE


def make_in_maps(mv, sc, pos, w_mv, w_s2mv, w_mv2s, w_s, b_s, batch_idx):
    wmm0, wmm, wsc = prep_weights(
        np.asarray(w_mv, np.float32), np.asarray(w_s2mv, np.float32),
        np.asarray(w_mv2s, np.float32), np.asarray(w_s, np.float32),
        np.asarray(b_s, np.float32))
    mv = np.ascontiguousarray(mv, np.float32).reshape(-1, 256)
    sc = np.ascontiguousarray(sc, np.float32)
    pos = np.ascontiguousarray(pos, np.float32)
    bidx = np.ascontiguousarray(batch_idx).astype(np.int64, copy=False)
    in_maps = []
    for c in range(NCORES):
        L = slice(c * NL_CORE, (c + 1) * NL_CORE)
        in_maps.append({
            "mv": np.ascontiguousarray(
                mv[L].reshape(NB_CORE, 32, 4, 4).transpose(0, 2, 1, 3)
            ).reshape(NB_CORE, 512),
            "sc": np.ascontiguousarray(sc[L]).reshape(NB_CORE, 32),
            "pos": np.ascontiguousarray(pos[L]).reshape(NB_CORE, 6),
            "wmm0": wmm0,
            "wmm": wmm,
            "wsc": wsc,
            "bidx": np.ascontiguousarray(bidx[L]).view(np.int32).reshape(128, 512),
        })
    return in_maps


def assemble(results):
    mv_n = np.concatenate(
        [r["mvn"].reshape(NB_CORE, 16, 16) for r in results], axis=0)
    sc_n = np.concatenate([r["scn"] for r in results], axis=0)
    centers = np.concatenate([r["cen"] for r in results], axis=0)
    batch_out = np.concatenate(
        [np.ascontiguousarray(r["bout"]).reshape(-1).view(np.int64)
         for r in results], axis=0)
    return mv_n, sc_n, centers, batch_out


def kernel(mv, sc, pos, w_mv, w_s2mv, w_mv2s, w_s, b_s, batch_idx):
    from concourse.bass_utils import run_bass_kernel_spmd

    nc = _get_nc()
    in_maps = make_in_maps(mv, sc, pos, w_mv, w_s2mv, w_mv2s, w_s, b_s, batch_idx)
    res = run_bass_kernel_spmd(nc, in_maps, core_ids=list(range(NCORES)))
    return assemble(res.results)
